# revision 15
# baseline (speedup 1.0000x reference)
"""Trainium2 Bass kernel: batched scaled-dot-product attention.

reference: out[b] = softmax(scale * x1[b] @ x2[b].T) @ x3[b]
shapes: x1,x2,x3 = [16, 2048, 128] fp32.

Sharding: B=16 batches data-parallel over 8 NeuronCores (2 batches/core).

Device algorithm (per batch, per q-half of 1024):
  for k-chunk in 16 (128 K-rows each):
    S^T[k, q]   = matmul(lhsT=K^T chunk, rhs=Q^T half)    TensorE float32r
    eS          = exp(scale * S^T)     PSUM -> SBUF f32r  ScalarE
    outT[dv,q] += matmul(lhsT=V chunk, rhs=eS)            TensorE accumulate
    softmax denominators: partial sums of eS chunks accumulated on
    GPSIMD (chunks 0-5) and DVE (chunks 6-15)
  sums = ones-matmul(acc_g) + ones-matmul(acc_v)  (PSUM accumulate, bcast to
  all partitions);  rcp = approx 1/sums (DVE);  out^T = outT * rcp -> DMA.

Host side does layout only: pre-transpose Q,K; interleave V rows; pre-round
inputs to float32r (e8m11, bit-identical to the device cast); transpose the
output back. All FLOPs run on device.
"""
import os
import sys
import types
import numpy as np
from contextlib import ExitStack

import concourse.bass as bass  # noqa: F401
from concourse import bacc
import concourse.mybir as mybir
import concourse.tile as tile
from concourse.tile_rust import add_dep_helper
import concourse.bass_utils as bass_utils
from concourse.bass_utils import run_bass_kernel_spmd

f32 = mybir.dt.float32
f32r = mybir.dt.float32r

B, SQ, SK, D = 16, 2048, 2048, 128
NCORES = 8
BPC = B // NCORES  # batches per core
KC = SK // 128     # k chunks
NH = 2             # q halves
HW_ = SQ // NH     # 1024
GP_CHUNKS = 6      # sums chunks 0..GP_CHUNKS-1 on GPSIMD, rest on DVE


def _patch_ldw_opt():
    """Enable walrus LDWEIGHTS optimization (background weight-buffer overlap)."""
    if getattr(bass_utils, "_ldw_patched", False):
        return
    if not bool(int(os.environ.get("KERNEL_LDWOPT", "1"))):
        return
    orig = bass_utils.run_command

    def patched(argv, **kw):
        if isinstance(argv, list):
            argv = [
                "--enable-ldw-opt=true" if a == "--enable-ldw-opt=false" else a
                for a in argv
            ]
        return orig(argv, **kw)

    bass_utils.run_command = patched
    bass_utils._ldw_patched = True


def _install_ntff_hook():
    """Register the axon NTFF profile hook (used only when tracing)."""
    try:
        from antenv import axon_hooks  # noqa: F401
        return
    except ImportError:
        pass
    try:
        m = types.ModuleType("antenv.axon_hooks")
        m._hook = None
        m.set_axon_ntff_profile_hook = lambda h: setattr(m, "_hook", h)
        m.get_axon_ntff_profile_hook = lambda: m._hook
        sys.modules["antenv.axon_hooks"] = m
        import antenv
        antenv.axon_hooks = m
        from trn_agent_boot.trn_boot import _ntff_profile_via_ctypes
        m._hook = _ntff_profile_via_ctypes("/opt/axon/libaxon_pjrt.so")
    except Exception:
        pass


def round_fp32r(x: np.ndarray) -> np.ndarray:
    """RNE round fp32 -> float32r (e8m11: drop low 12 mantissa bits)."""
    u = np.ascontiguousarray(x, dtype=np.float32).view(np.uint32).astype(np.uint64)
    keep = 12
    lsb = (u >> keep) & 1
    bias = (1 << (keep - 1)) - 1 + lsb
    r = ((u + bias) & ~np.uint64((1 << keep) - 1)).astype(np.uint32)
    return r.view(np.float32)


def build(scale: float):
    _patch_ldw_opt()
    nc = bacc.Bacc("TRN2", target_bir_lowering=False, debug=False)
    qt = nc.dram_tensor("qt", [BPC, 128, SQ], f32r, kind="ExternalInput")
    kt = nc.dram_tensor("kt", [BPC, 128, SK], f32r, kind="ExternalInput")
    vv = nc.dram_tensor("v", [BPC, 128, SK], f32r, kind="ExternalInput")
    ones = nc.dram_tensor("ones", [128, 128], f32r, kind="ExternalInput")
    oo = nc.dram_tensor("o", [BPC, 128, SQ], f32, kind="ExternalOutput")

    Exp = mybir.ActivationFunctionType.Exp

    with tile.TileContext(nc) as tc, ExitStack() as ctx:
        inp = ctx.enter_context(tc.tile_pool(name="inp", bufs=2))
        es_pool = ctx.enter_context(tc.tile_pool(name="es", bufs=18))
        acc_pool = ctx.enter_context(tc.tile_pool(name="acc", bufs=2))
        out_pool = ctx.enter_context(tc.tile_pool(name="out", bufs=2))
        cpool = ctx.enter_context(tc.tile_pool(name="const", bufs=1))
        psS = ctx.enter_context(tc.tile_pool(name="psS", bufs=2, space="PSUM"))
        psO = ctx.enter_context(tc.tile_pool(name="psO", bufs=2, space="PSUM"))

        ones_sb = cpool.tile([128, 128], f32r, tag="ones")
        nc.sync.dma_start(ones_sb[:], ones.ap())

        def make_tail(ou, srcs, osl, out_dma):
            """Deferred per-half epilogue: sums bcast + recip + normalize.

            sums = ones @ (acc_gr + acc_vr + es15) accumulated in PSUM; es15
            itself is the third source so no add ever waits on the last exp.
            Emitted shortly into the NEXT half; `anchor` orders the matmuls
            behind that point in the PE stream."""
            def emit(anchor):
                ps_b = psO.tile([128, HW_], f32, tag="psO")
                n_src = len(srcs)
                for si, srct in enumerate(srcs):
                    for j in range(HW_ // 512):
                        mm = nc.tensor.matmul(
                            ps_b[:, j * 512:(j + 1) * 512],
                            ones_sb[:],
                            srct[:, j * 512:(j + 1) * 512],
                            start=(si == 0), stop=(si == n_src - 1),
                        )
                        if anchor is not None:
                            add_dep_helper(mm.ins, anchor.ins, sync=False,
                                           reason="sums-mm deferred")
                # recip + normalize + store pipelined in 512-wide blocks
                rcp = acc_pool.tile([128, HW_], f32, tag="rcp")
                for j in range(HW_ // 512):
                    jj = slice(j * 512, (j + 1) * 512)
                    nc.vector.reciprocal_approx_fast(
                        out=rcp[:, jj], in_=ps_b[:, jj])
                    nc.vector.tensor_mul(osl[:, jj], ou[:, jj], rcp[:, jj])
                    out_dma(j)
            return emit

        pending_tail = None

        def flush_tail(anchor):
            nonlocal pending_tail
            if pending_tail is not None:
                pending_tail(anchor)
                pending_tail = None

        for b in range(BPC):
            qt_sb = inp.tile([128, SQ], f32r, tag="qt")
            kt_sb = inp.tile([128, SK], f32r, tag="kt")
            v_sb = inp.tile([128, SK], f32r, tag="v")
            if b == 0:
                # cold start: critical-first fine-grained loads so chunk 0
                # can begin while the rest streams in
                qa, ka, va = qt.ap()[b], kt.ap()[b], vv.ap()[b]
                nc.sync.dma_start(kt_sb[:, 0:128], ka[:, 0:128])
                nc.sync.dma_start(qt_sb[:, 0:512], qa[:, 0:512])
                nc.sync.dma_start(qt_sb[:, 512:HW_], qa[:, 512:HW_])
                nc.sync.dma_start(kt_sb[:, 128:256], ka[:, 128:256])
                nc.sync.dma_start(v_sb[:, 0:128], va[:, 0:128])
                nc.sync.dma_start(kt_sb[:, 256:512], ka[:, 256:512])
                nc.sync.dma_start(v_sb[:, 128:512], va[:, 128:512])
                G = 4 * 128
                for g in range(1, KC * 128 // G):
                    sl = slice(g * G, (g + 1) * G)
                    nc.sync.dma_start(kt_sb[:, sl], ka[:, sl])
                    nc.sync.dma_start(v_sb[:, sl], va[:, sl])
                nc.sync.dma_start(qt_sb[:, HW_:SQ], qa[:, HW_:SQ])
            else:
                # prefetched during batch 0 compute: one DMA per tensor keeps
                # the wait structure on batch-1's first matmuls minimal
                nc.sync.dma_start(qt_sb[:], qt.ap()[b])
                nc.sync.dma_start(kt_sb[:], kt.ap()[b])
                nc.sync.dma_start(v_sb[:], vv.ap()[b])
            ot_sb = out_pool.tile([128, SQ], f32, tag="ot")

            for h in range(NH):
                q0 = h * HW_
                ps_o = psO.tile([128, HW_], f32, tag="psO")
                acc_g = acc_pool.tile([128, HW_], f32, tag="accg")
                acc_gr = acc_pool.tile([128, HW_], f32r, tag="accgr")
                acc_v = acc_pool.tile([128, HW_], f32, tag="accv")
                acc_vr = acc_pool.tile([128, HW_], f32r, tag="accvr")
                es_prev = None
                last_qk = None
                raw_es = []
                for k in range(KC):
                    ps_s = psS.tile([128, HW_], f32, tag="S")
                    for j in range(HW_ // 512):
                        last_qk = nc.tensor.matmul(
                            ps_s[:, j * 512:(j + 1) * 512],
                            kt_sb[:, k * 128:(k + 1) * 128],
                            qt_sb[:, q0 + j * 512:q0 + (j + 1) * 512],
                            start=True, stop=True,
                        )
                    if k == 3:
                        flush_tail(last_qk)
                    es = es_pool.tile([128, HW_], f32r, tag="es")
                    nc.scalar.activation(es[:], ps_s[:], Exp, scale=scale)
                    for j in range(HW_ // 512):
                        nc.tensor.matmul(
                            ps_o[:, j * 512:(j + 1) * 512],
                            v_sb[:, k * 128:(k + 1) * 128],
                            es[:, j * 512:(j + 1) * 512],
                            start=(k == 0), stop=(k == KC - 1),
                        )
                    esf = es[:].bitcast(f32)
                    # softmax-denominator partials:
                    #   chunks 0-4  -> GPSIMD (front-loaded; last add f32r out)
                    #   chunks 5-14 -> DVE (last add f32r out)
                    #   chunk 15    -> fed raw (f32r) to the sums matmul
                    if k == 0 or k == GP_CHUNKS:
                        pass
                    elif k < GP_CHUNKS:
                        if k == 1:
                            nc.gpsimd.tensor_add(acc_g[:], es_prev, esf)
                        elif k == GP_CHUNKS - 1:
                            nc.gpsimd.tensor_add(acc_gr[:], acc_g[:], esf)
                        else:
                            nc.gpsimd.tensor_add(acc_g[:], acc_g[:], esf)
                    elif k == GP_CHUNKS + 1:
                        nc.vector.tensor_add(acc_v[:], es_prev, esf)
                    elif k == KC - 4:
                        nc.vector.tensor_add(acc_vr[:], acc_v[:], esf)
                    elif k >= KC - 3:
                        raw_es.append(es)
                        if k == KC - 1:
                            # drain the PV accumulator promptly: frees the psO
                            # slot that the deferred sums-matmul will reuse
                            ou = acc_pool.tile([128, HW_], f32, tag="ou")
                            nc.vector.tensor_copy(ou[:], ps_o[:])
                    else:
                        nc.vector.tensor_add(acc_v[:], acc_v[:], esf)
                    es_prev = esf

                dram_half = oo.ap()[b][:, q0:q0 + HW_]
                tile_half = ot_sb[:, q0:q0 + HW_]

                def out_dma(j, dram_half=dram_half, tile_half=tile_half):
                    jj = slice(j * 512, (j + 1) * 512)
                    nc.sync.dma_start(dram_half[:, jj], tile_half[:, jj])

                pending_tail = make_tail(
                    ou, [acc_gr, acc_vr] + [t[:] for t in raw_es],
                    tile_half, out_dma)

        # final half's epilogue + last output store
        flush_tail(None)

    nc.compile()
    return nc


_BUILD_CACHE = {}


def _get_nc(scale: float):
    key = round(float(scale), 9)
    if key not in _BUILD_CACHE:
        _BUILD_CACHE[key] = build(float(scale))
    return _BUILD_CACHE[key]


def kernel(x1, x2, x3, x4=None, scale_factor=None, **_ignored):
    x1 = np.asarray(x1, dtype=np.float32)
    x2 = np.asarray(x2, dtype=np.float32)
    x3 = np.asarray(x3, dtype=np.float32)
    scale = float(np.asarray(scale_factor).reshape(-1)[0])

    # host prep: transpose Q,K to [d, s]; interleave V rows to [p, c*d]; round f32r
    qt = round_fp32r(x1.transpose(0, 2, 1))                     # [B, 128, SQ]
    kt = round_fp32r(x2.transpose(0, 2, 1))                     # [B, 128, SK]
    v = round_fp32r(
        x3.reshape(B, KC, 128, D).transpose(0, 2, 1, 3).reshape(B, 128, KC * D)
    )                                                           # [B, 128, SK]
    ones = np.ones((128, 128), dtype=np.float32)

    nc = _get_nc(scale)
    in_maps = []
    for c in range(NCORES):
        s = slice(c * BPC, (c + 1) * BPC)
        in_maps.append({
            "qt": np.ascontiguousarray(qt[s]),
            "kt": np.ascontiguousarray(kt[s]),
            "v": np.ascontiguousarray(v[s]),
            "ones": ones,
        })

    trace = bool(int(os.environ.get("KERNEL_TRACE", "0")))
    kwargs = {}
    if trace:
        _install_ntff_hook()
        if bool(int(os.environ.get("KERNEL_TRACE_ALL", "0"))):
            os.environ["BASS_PERFETTO_PROFILE_ALL_CORES"] = "1"
        kwargs = dict(trace=True, trace_kwargs={"title": "attention"})
    res = run_bass_kernel_spmd(nc, in_maps, core_ids=list(range(NCORES)), **kwargs)
    if trace:
        kernel.last_exec_ns = res.exec_time_ns
        kernel.last_trace = res.instructions_and_trace
        kernel.last_mean_exec_ns = res.mean_exec_time_ns

    outT = np.stack([r["o"] for r in res.results])              # [8, BPC, 128, SQ]
    out = outT.reshape(B, 128, SQ).transpose(0, 2, 1)           # [B, SQ, 128]
    return np.ascontiguousarray(out, dtype=np.float32)


kernel.last_exec_ns = None
kernel.last_trace = None
kernel.last_mean_exec_ns = None


# revision 16
# speedup vs baseline: 1.0070x; 1.0070x over previous
"""Trainium2 Bass kernel: batched scaled-dot-product attention.

reference: out[b] = softmax(scale * x1[b] @ x2[b].T) @ x3[b]
shapes: x1,x2,x3 = [16, 2048, 128] fp32.

Sharding: B=16 batches data-parallel over 8 NeuronCores (2 batches/core).

Device algorithm (per batch, per q-half of 1024):
  for k-chunk in 16 (128 K-rows each):
    S^T[k, q]   = matmul(lhsT=K^T chunk, rhs=Q^T half)    TensorE float32r
    eS          = exp(scale * S^T)     PSUM -> SBUF f32r  ScalarE
    outT[dv,q] += matmul(lhsT=V chunk, rhs=eS)            TensorE accumulate
    softmax denominators: partial sums of eS chunks accumulated on
    GPSIMD (chunks 0-5) and DVE (chunks 6-15)
  sums = ones-matmul(acc_g) + ones-matmul(acc_v)  (PSUM accumulate, bcast to
  all partitions);  rcp = approx 1/sums (DVE);  out^T = outT * rcp -> DMA.

Host side does layout only: pre-transpose Q,K; interleave V rows; pre-round
inputs to float32r (e8m11, bit-identical to the device cast); transpose the
output back. All FLOPs run on device.
"""
import os
import sys
import types
import numpy as np
from contextlib import ExitStack

import concourse.bass as bass  # noqa: F401
from concourse import bacc
import concourse.mybir as mybir
import concourse.tile as tile
from concourse.tile_rust import add_dep_helper
import concourse.bass_utils as bass_utils
from concourse.bass_utils import run_bass_kernel_spmd

f32 = mybir.dt.float32
f32r = mybir.dt.float32r

B, SQ, SK, D = 16, 2048, 2048, 128
NCORES = 8
BPC = B // NCORES  # batches per core
KC = SK // 128     # k chunks
NH = 2             # q halves
HW_ = SQ // NH     # 1024
GP_CHUNKS = 6      # sums chunks 0..GP_CHUNKS-1 on GPSIMD, rest on DVE


def _patch_ldw_opt():
    """Enable walrus LDWEIGHTS optimization (background weight-buffer overlap)."""
    if getattr(bass_utils, "_ldw_patched", False):
        return
    if not bool(int(os.environ.get("KERNEL_LDWOPT", "1"))):
        return
    orig = bass_utils.run_command

    def patched(argv, **kw):
        if isinstance(argv, list):
            argv = [
                "--enable-ldw-opt=true" if a == "--enable-ldw-opt=false" else a
                for a in argv
            ]
        return orig(argv, **kw)

    bass_utils.run_command = patched
    bass_utils._ldw_patched = True


def _install_ntff_hook():
    """Register the axon NTFF profile hook (used only when tracing)."""
    try:
        from antenv import axon_hooks  # noqa: F401
        return
    except ImportError:
        pass
    try:
        m = types.ModuleType("antenv.axon_hooks")
        m._hook = None
        m.set_axon_ntff_profile_hook = lambda h: setattr(m, "_hook", h)
        m.get_axon_ntff_profile_hook = lambda: m._hook
        sys.modules["antenv.axon_hooks"] = m
        import antenv
        antenv.axon_hooks = m
        from trn_agent_boot.trn_boot import _ntff_profile_via_ctypes
        m._hook = _ntff_profile_via_ctypes("/opt/axon/libaxon_pjrt.so")
    except Exception:
        pass


def round_fp32r(x: np.ndarray) -> np.ndarray:
    """RNE round fp32 -> float32r (e8m11: drop low 12 mantissa bits)."""
    u = np.ascontiguousarray(x, dtype=np.float32).view(np.uint32).astype(np.uint64)
    keep = 12
    lsb = (u >> keep) & 1
    bias = (1 << (keep - 1)) - 1 + lsb
    r = ((u + bias) & ~np.uint64((1 << keep) - 1)).astype(np.uint32)
    return r.view(np.float32)


def build(scale: float):
    _patch_ldw_opt()
    nc = bacc.Bacc("TRN2", target_bir_lowering=False, debug=False)
    qt = nc.dram_tensor("qt", [BPC, 128, SQ], f32r, kind="ExternalInput")
    kt = nc.dram_tensor("kt", [BPC, 128, SK], f32r, kind="ExternalInput")
    vv = nc.dram_tensor("v", [BPC, 128, SK], f32r, kind="ExternalInput")
    ones = nc.dram_tensor("ones", [128, 128], f32r, kind="ExternalInput")
    oo = nc.dram_tensor("o", [BPC, 128, SQ], f32, kind="ExternalOutput")

    Exp = mybir.ActivationFunctionType.Exp

    with tile.TileContext(nc) as tc, ExitStack() as ctx:
        inp = ctx.enter_context(tc.tile_pool(name="inp", bufs=2))
        es_pool = ctx.enter_context(tc.tile_pool(name="es", bufs=18))
        acc_pool = ctx.enter_context(tc.tile_pool(name="acc", bufs=2))
        out_pool = ctx.enter_context(tc.tile_pool(name="out", bufs=2))
        cpool = ctx.enter_context(tc.tile_pool(name="const", bufs=1))
        psS = ctx.enter_context(tc.tile_pool(name="psS", bufs=2, space="PSUM"))
        psO = ctx.enter_context(tc.tile_pool(name="psO", bufs=2, space="PSUM"))

        ones_sb = cpool.tile([128, 128], f32r, tag="ones")
        nc.sync.dma_start(ones_sb[:], ones.ap())

        def make_tail(ps_o, srcs, osl, out_dma):
            """Deferred per-half epilogue: sums bcast + recip + normalize.

            sums = ones @ (acc_gr + acc_vr + es15) accumulated in PSUM; es15
            itself is the third source so no add ever waits on the last exp.
            Emitted shortly into the NEXT half; `anchor` orders the matmuls
            behind that point in the PE stream."""
            def emit(anchor):
                ps_b = psS.tile([128, HW_], f32, tag="S")
                n_src = len(srcs)
                for si, srct in enumerate(srcs):
                    for j in range(HW_ // 512):
                        mm = nc.tensor.matmul(
                            ps_b[:, j * 512:(j + 1) * 512],
                            ones_sb[:],
                            srct[:, j * 512:(j + 1) * 512],
                            start=(si == 0), stop=(si == n_src - 1),
                        )
                        if anchor is not None:
                            add_dep_helper(mm.ins, anchor.ins, sync=False,
                                           reason="sums-mm deferred")
                # recip + fused normalize + store pipelined in 512-wide blocks
                rcp = acc_pool.tile([128, HW_], f32, tag="rcp")
                for j in range(HW_ // 512):
                    jj = slice(j * 512, (j + 1) * 512)
                    nc.vector.reciprocal_approx_fast(
                        out=rcp[:, jj], in_=ps_b[:, jj])
                    nc.vector.scalar_tensor_tensor(
                        osl[:, jj], ps_o[:, jj], 1.0, rcp[:, jj],
                        op0=mybir.AluOpType.mult, op1=mybir.AluOpType.mult,
                    )
                    out_dma(j)
            return emit

        pending_tail = None

        def flush_tail(anchor):
            nonlocal pending_tail
            if pending_tail is not None:
                pending_tail(anchor)
                pending_tail = None

        for b in range(BPC):
            qt_sb = inp.tile([128, SQ], f32r, tag="qt")
            kt_sb = inp.tile([128, SK], f32r, tag="kt")
            v_sb = inp.tile([128, SK], f32r, tag="v")
            if b == 0:
                # cold start: critical-first fine-grained loads so chunk 0
                # can begin while the rest streams in
                qa, ka, va = qt.ap()[b], kt.ap()[b], vv.ap()[b]
                nc.sync.dma_start(kt_sb[:, 0:128], ka[:, 0:128])
                nc.sync.dma_start(qt_sb[:, 0:512], qa[:, 0:512])
                nc.sync.dma_start(qt_sb[:, 512:HW_], qa[:, 512:HW_])
                nc.sync.dma_start(kt_sb[:, 128:256], ka[:, 128:256])
                nc.sync.dma_start(v_sb[:, 0:128], va[:, 0:128])
                nc.sync.dma_start(kt_sb[:, 256:512], ka[:, 256:512])
                nc.sync.dma_start(v_sb[:, 128:512], va[:, 128:512])
                G = 4 * 128
                for g in range(1, KC * 128 // G):
                    sl = slice(g * G, (g + 1) * G)
                    nc.sync.dma_start(kt_sb[:, sl], ka[:, sl])
                    nc.sync.dma_start(v_sb[:, sl], va[:, sl])
                nc.sync.dma_start(qt_sb[:, HW_:SQ], qa[:, HW_:SQ])
            else:
                # prefetched during batch 0 compute: one DMA per tensor keeps
                # the wait structure on batch-1's first matmuls minimal
                nc.sync.dma_start(qt_sb[:], qt.ap()[b])
                nc.sync.dma_start(kt_sb[:], kt.ap()[b])
                nc.sync.dma_start(v_sb[:], vv.ap()[b])
            ot_sb = out_pool.tile([128, SQ], f32, tag="ot")

            for h in range(NH):
                q0 = h * HW_
                ps_o = psO.tile([128, HW_], f32, tag="psO")
                acc_g = acc_pool.tile([128, HW_], f32, tag="accg")
                acc_gr = acc_pool.tile([128, HW_], f32r, tag="accgr")
                acc_v = acc_pool.tile([128, HW_], f32, tag="accv")
                acc_vr = acc_pool.tile([128, HW_], f32r, tag="accvr")
                es_prev = None
                last_qk = None
                raw_es = []
                for k in range(KC):
                    ps_s = psS.tile([128, HW_], f32, tag="S")
                    for j in range(HW_ // 512):
                        last_qk = nc.tensor.matmul(
                            ps_s[:, j * 512:(j + 1) * 512],
                            kt_sb[:, k * 128:(k + 1) * 128],
                            qt_sb[:, q0 + j * 512:q0 + (j + 1) * 512],
                            start=True, stop=True,
                        )
                    if k == 3:
                        flush_tail(last_qk)
                    es = es_pool.tile([128, HW_], f32r, tag="es")
                    nc.scalar.activation(es[:], ps_s[:], Exp, scale=scale)
                    for j in range(HW_ // 512):
                        nc.tensor.matmul(
                            ps_o[:, j * 512:(j + 1) * 512],
                            v_sb[:, k * 128:(k + 1) * 128],
                            es[:, j * 512:(j + 1) * 512],
                            start=(k == 0), stop=(k == KC - 1),
                        )
                    esf = es[:].bitcast(f32)
                    # softmax-denominator partials:
                    #   chunks 0-4  -> GPSIMD (front-loaded; last add f32r out)
                    #   chunks 5-14 -> DVE (last add f32r out)
                    #   chunk 15    -> fed raw (f32r) to the sums matmul
                    if k == 0 or k == GP_CHUNKS:
                        pass
                    elif k < GP_CHUNKS:
                        if k == 1:
                            nc.gpsimd.tensor_add(acc_g[:], es_prev, esf)
                        elif k == GP_CHUNKS - 1:
                            nc.gpsimd.tensor_add(acc_gr[:], acc_g[:], esf)
                        else:
                            nc.gpsimd.tensor_add(acc_g[:], acc_g[:], esf)
                    elif k == GP_CHUNKS + 1:
                        nc.vector.tensor_add(acc_v[:], es_prev, esf)
                    elif k == KC - 4:
                        nc.vector.tensor_add(acc_vr[:], acc_v[:], esf)
                    elif k >= KC - 3:
                        raw_es.append(es)
                    else:
                        nc.vector.tensor_add(acc_v[:], acc_v[:], esf)
                    es_prev = esf

                dram_half = oo.ap()[b][:, q0:q0 + HW_]
                tile_half = ot_sb[:, q0:q0 + HW_]

                def out_dma(j, dram_half=dram_half, tile_half=tile_half):
                    jj = slice(j * 512, (j + 1) * 512)
                    nc.sync.dma_start(dram_half[:, jj], tile_half[:, jj])

                pending_tail = make_tail(
                    ps_o, [acc_gr, acc_vr] + [t[:] for t in raw_es],
                    tile_half, out_dma)

        # final half's epilogue + last output store
        flush_tail(None)

    nc.compile()
    return nc


_BUILD_CACHE = {}


def _get_nc(scale: float):
    key = round(float(scale), 9)
    if key not in _BUILD_CACHE:
        _BUILD_CACHE[key] = build(float(scale))
    return _BUILD_CACHE[key]


def kernel(x1, x2, x3, x4=None, scale_factor=None, **_ignored):
    x1 = np.asarray(x1, dtype=np.float32)
    x2 = np.asarray(x2, dtype=np.float32)
    x3 = np.asarray(x3, dtype=np.float32)
    scale = float(np.asarray(scale_factor).reshape(-1)[0])

    # host prep: transpose Q,K to [d, s]; interleave V rows to [p, c*d]; round f32r
    qt = round_fp32r(x1.transpose(0, 2, 1))                     # [B, 128, SQ]
    kt = round_fp32r(x2.transpose(0, 2, 1))                     # [B, 128, SK]
    v = round_fp32r(
        x3.reshape(B, KC, 128, D).transpose(0, 2, 1, 3).reshape(B, 128, KC * D)
    )                                                           # [B, 128, SK]
    ones = np.ones((128, 128), dtype=np.float32)

    nc = _get_nc(scale)
    in_maps = []
    for c in range(NCORES):
        s = slice(c * BPC, (c + 1) * BPC)
        in_maps.append({
            "qt": np.ascontiguousarray(qt[s]),
            "kt": np.ascontiguousarray(kt[s]),
            "v": np.ascontiguousarray(v[s]),
            "ones": ones,
        })

    trace = bool(int(os.environ.get("KERNEL_TRACE", "0")))
    kwargs = {}
    if trace:
        _install_ntff_hook()
        if bool(int(os.environ.get("KERNEL_TRACE_ALL", "0"))):
            os.environ["BASS_PERFETTO_PROFILE_ALL_CORES"] = "1"
        kwargs = dict(trace=True, trace_kwargs={"title": "attention"})
    res = run_bass_kernel_spmd(nc, in_maps, core_ids=list(range(NCORES)), **kwargs)
    if trace:
        kernel.last_exec_ns = res.exec_time_ns
        kernel.last_trace = res.instructions_and_trace
        kernel.last_mean_exec_ns = res.mean_exec_time_ns

    outT = np.stack([r["o"] for r in res.results])              # [8, BPC, 128, SQ]
    out = outT.reshape(B, 128, SQ).transpose(0, 2, 1)           # [B, SQ, 128]
    return np.ascontiguousarray(out, dtype=np.float32)


kernel.last_exec_ns = None
kernel.last_trace = None
kernel.last_mean_exec_ns = None


# revision 17
# speedup vs baseline: 1.0361x; 1.0289x over previous
"""Trainium2 Bass kernel: batched scaled-dot-product attention.

reference: out[b] = softmax(scale * x1[b] @ x2[b].T) @ x3[b]
shapes: x1,x2,x3 = [16, 2048, 128] fp32.

Sharding: B=16 batches data-parallel over 8 NeuronCores (2 batches/core).

Device algorithm (per batch, per q-half of 1024):
  for k-chunk in 16 (128 K-rows each):
    S^T[k, q]   = matmul(lhsT=K^T chunk, rhs=Q^T half)    TensorE float32r
    eS          = exp(scale * S^T)     PSUM -> SBUF f32r  ScalarE
    outT[dv,q] += matmul(lhsT=V chunk, rhs=eS)            TensorE accumulate
    softmax denominators: partial sums of eS chunks accumulated on
    GPSIMD (chunks 0-5) and DVE (chunks 6-15)
  sums = ones-matmul(acc_g) + ones-matmul(acc_v)  (PSUM accumulate, bcast to
  all partitions);  rcp = approx 1/sums (DVE);  out^T = outT * rcp -> DMA.

Host side does layout only: pre-transpose Q,K; interleave V rows; pre-round
inputs to float32r (e8m11, bit-identical to the device cast); transpose the
output back. All FLOPs run on device.
"""
import os
import sys
import types
import numpy as np
from contextlib import ExitStack

import concourse.bass as bass  # noqa: F401
from concourse import bacc
import concourse.mybir as mybir
import concourse.tile as tile
from concourse.tile_rust import add_dep_helper
import concourse.bass_utils as bass_utils
from concourse.bass_utils import run_bass_kernel_spmd

f32 = mybir.dt.float32
f32r = mybir.dt.float32r

B, SQ, SK, D = 16, 2048, 2048, 128
NCORES = 8
BPC = B // NCORES  # batches per core
KC = SK // 128     # k chunks
NH = 2             # q halves
HW_ = SQ // NH     # 1024
GP_CHUNKS = 6      # sums chunks 0..GP_CHUNKS-1 on GPSIMD, rest on DVE


def _patch_ldw_opt():
    """Enable walrus LDWEIGHTS optimization (background weight-buffer overlap)."""
    if getattr(bass_utils, "_ldw_patched", False):
        return
    if not bool(int(os.environ.get("KERNEL_LDWOPT", "1"))):
        return
    orig = bass_utils.run_command

    def patched(argv, **kw):
        if isinstance(argv, list):
            argv = [
                "--enable-ldw-opt=true" if a == "--enable-ldw-opt=false" else a
                for a in argv
            ]
        return orig(argv, **kw)

    bass_utils.run_command = patched
    bass_utils._ldw_patched = True


def _install_ntff_hook():
    """Register the axon NTFF profile hook (used only when tracing)."""
    try:
        from antenv import axon_hooks  # noqa: F401
        return
    except ImportError:
        pass
    try:
        m = types.ModuleType("antenv.axon_hooks")
        m._hook = None
        m.set_axon_ntff_profile_hook = lambda h: setattr(m, "_hook", h)
        m.get_axon_ntff_profile_hook = lambda: m._hook
        sys.modules["antenv.axon_hooks"] = m
        import antenv
        antenv.axon_hooks = m
        from trn_agent_boot.trn_boot import _ntff_profile_via_ctypes
        m._hook = _ntff_profile_via_ctypes("/opt/axon/libaxon_pjrt.so")
    except Exception:
        pass


def round_fp32r(x: np.ndarray) -> np.ndarray:
    """RNE round fp32 -> float32r (e8m11: drop low 12 mantissa bits)."""
    u = np.ascontiguousarray(x, dtype=np.float32).view(np.uint32).astype(np.uint64)
    keep = 12
    lsb = (u >> keep) & 1
    bias = (1 << (keep - 1)) - 1 + lsb
    r = ((u + bias) & ~np.uint64((1 << keep) - 1)).astype(np.uint32)
    return r.view(np.float32)


def build(scale: float):
    _patch_ldw_opt()
    nc = bacc.Bacc("TRN2", target_bir_lowering=False, debug=False)
    qt = nc.dram_tensor("qt", [BPC, 128, SQ], f32r, kind="ExternalInput")
    kt = nc.dram_tensor("kt", [BPC, 128, SK], f32r, kind="ExternalInput")
    vv = nc.dram_tensor("v", [BPC, 128, SK], f32r, kind="ExternalInput")
    ones = nc.dram_tensor("ones", [128, 128], f32r, kind="ExternalInput")
    oo = nc.dram_tensor("o", [BPC, 128, SQ], f32, kind="ExternalOutput")

    Exp = mybir.ActivationFunctionType.Exp

    with tile.TileContext(nc) as tc, ExitStack() as ctx:
        inp = ctx.enter_context(tc.tile_pool(name="inp", bufs=2))
        es_pool = ctx.enter_context(tc.tile_pool(name="es", bufs=18))
        acc_pool = ctx.enter_context(tc.tile_pool(name="acc", bufs=2))
        out_pool = ctx.enter_context(tc.tile_pool(name="out", bufs=2))
        cpool = ctx.enter_context(tc.tile_pool(name="const", bufs=1))
        psS = ctx.enter_context(tc.tile_pool(name="psS", bufs=2, space="PSUM"))
        psO = ctx.enter_context(tc.tile_pool(name="psO", bufs=2, space="PSUM"))

        ones_sb = cpool.tile([128, 128], f32r, tag="ones")
        nc.sync.dma_start(ones_sb[:], ones.ap())

        def make_tail(ps_o, srcs, osl, out_dma):
            """Deferred per-half epilogue: sums bcast + recip + normalize.

            sums = ones @ (acc_gr + acc_vr + es15) accumulated in PSUM; es15
            itself is the third source so no add ever waits on the last exp.
            Emitted shortly into the NEXT half; `anchor` orders the matmuls
            behind that point in the PE stream."""
            def emit(anchor):
                ps_b = psS.tile([128, HW_], f32, tag="S")
                n_src = len(srcs)
                for si, srct in enumerate(srcs):
                    for j in range(HW_ // 512):
                        mm = nc.tensor.matmul(
                            ps_b[:, j * 512:(j + 1) * 512],
                            ones_sb[:],
                            srct[:, j * 512:(j + 1) * 512],
                            start=(si == 0), stop=(si == n_src - 1),
                        )
                        if anchor is not None:
                            add_dep_helper(mm.ins, anchor.ins, sync=False,
                                           reason="sums-mm deferred")
                rcp = acc_pool.tile([128, HW_], f32, tag="rcp")
                nc.vector.reciprocal_approx_fast(out=rcp[:], in_=ps_b[:])
                # fused normalize osl = (ps_o * 1.0) * rcp in 512-wide blocks
                # so each output DMA starts as soon as its half is ready
                for j in range(HW_ // 512):
                    jj = slice(j * 512, (j + 1) * 512)
                    nc.vector.scalar_tensor_tensor(
                        osl[:, jj], ps_o[:, jj], 1.0, rcp[:, jj],
                        op0=mybir.AluOpType.mult, op1=mybir.AluOpType.mult,
                    )
                    out_dma(j)
            return emit

        pending_tail = None

        def flush_tail(anchor):
            nonlocal pending_tail
            if pending_tail is not None:
                pending_tail(anchor)
                pending_tail = None

        for b in range(BPC):
            qt_sb = inp.tile([128, SQ], f32r, tag="qt")
            kt_sb = inp.tile([128, SK], f32r, tag="kt")
            v_sb = inp.tile([128, SK], f32r, tag="v")
            if b == 0:
                # cold start: critical-first fine-grained loads so chunk 0
                # can begin while the rest streams in
                qa, ka, va = qt.ap()[b], kt.ap()[b], vv.ap()[b]
                nc.sync.dma_start(kt_sb[:, 0:128], ka[:, 0:128])
                nc.sync.dma_start(qt_sb[:, 0:512], qa[:, 0:512])
                nc.sync.dma_start(qt_sb[:, 512:HW_], qa[:, 512:HW_])
                nc.sync.dma_start(kt_sb[:, 128:256], ka[:, 128:256])
                nc.sync.dma_start(v_sb[:, 0:128], va[:, 0:128])
                nc.sync.dma_start(kt_sb[:, 256:512], ka[:, 256:512])
                nc.sync.dma_start(v_sb[:, 128:512], va[:, 128:512])
                G = 4 * 128
                for g in range(1, KC * 128 // G):
                    sl = slice(g * G, (g + 1) * G)
                    nc.sync.dma_start(kt_sb[:, sl], ka[:, sl])
                    nc.sync.dma_start(v_sb[:, sl], va[:, sl])
                nc.sync.dma_start(qt_sb[:, HW_:SQ], qa[:, HW_:SQ])
            else:
                # prefetched during batch 0 compute: one DMA per tensor keeps
                # the wait structure on batch-1's first matmuls minimal
                nc.sync.dma_start(qt_sb[:], qt.ap()[b])
                nc.sync.dma_start(kt_sb[:], kt.ap()[b])
                nc.sync.dma_start(v_sb[:], vv.ap()[b])
            ot_sb = out_pool.tile([128, SQ], f32, tag="ot")

            for h in range(NH):
                q0 = h * HW_
                ps_o = psO.tile([128, HW_], f32, tag="psO")
                acc_g = acc_pool.tile([128, HW_], f32, tag="accg")
                acc_gr = acc_pool.tile([128, HW_], f32r, tag="accgr")
                acc_v = acc_pool.tile([128, HW_], f32, tag="accv")
                acc_vr = acc_pool.tile([128, HW_], f32r, tag="accvr")
                es_prev = None
                last_qk = None
                raw_es = []
                for k in range(KC):
                    ps_s = psS.tile([128, HW_], f32, tag="S")
                    for j in range(HW_ // 512):
                        last_qk = nc.tensor.matmul(
                            ps_s[:, j * 512:(j + 1) * 512],
                            kt_sb[:, k * 128:(k + 1) * 128],
                            qt_sb[:, q0 + j * 512:q0 + (j + 1) * 512],
                            start=True, stop=True,
                        )
                    if k == 3:
                        flush_tail(last_qk)
                    es = es_pool.tile([128, HW_], f32r, tag="es")
                    nc.scalar.activation(es[:], ps_s[:], Exp, scale=scale)
                    for j in range(HW_ // 512):
                        nc.tensor.matmul(
                            ps_o[:, j * 512:(j + 1) * 512],
                            v_sb[:, k * 128:(k + 1) * 128],
                            es[:, j * 512:(j + 1) * 512],
                            start=(k == 0), stop=(k == KC - 1),
                        )
                    esf = es[:].bitcast(f32)
                    # softmax-denominator partials:
                    #   chunks 0-4  -> GPSIMD (front-loaded; last add f32r out)
                    #   chunks 5-14 -> DVE (last add f32r out)
                    #   chunk 15    -> fed raw (f32r) to the sums matmul
                    if k == 0 or k == GP_CHUNKS:
                        pass
                    elif k < GP_CHUNKS:
                        if k == 1:
                            nc.gpsimd.tensor_add(acc_g[:], es_prev, esf)
                        elif k == GP_CHUNKS - 1:
                            nc.gpsimd.tensor_add(acc_gr[:], acc_g[:], esf)
                        else:
                            nc.gpsimd.tensor_add(acc_g[:], acc_g[:], esf)
                    elif k == GP_CHUNKS + 1:
                        nc.vector.tensor_add(acc_v[:], es_prev, esf)
                    elif k == KC - 4:
                        nc.vector.tensor_add(acc_vr[:], acc_v[:], esf)
                    elif k >= KC - 3:
                        raw_es.append(es)
                    else:
                        nc.vector.tensor_add(acc_v[:], acc_v[:], esf)
                    es_prev = esf

                dram_half = oo.ap()[b][:, q0:q0 + HW_]
                tile_half = ot_sb[:, q0:q0 + HW_]

                def out_dma(j, dram_half=dram_half, tile_half=tile_half):
                    jj = slice(j * 512, (j + 1) * 512)
                    nc.sync.dma_start(dram_half[:, jj], tile_half[:, jj])

                pending_tail = make_tail(
                    ps_o, [acc_gr, acc_vr] + [t[:] for t in raw_es],
                    tile_half, out_dma)

        # final half's epilogue + last output store
        flush_tail(None)

    nc.compile()
    return nc


_BUILD_CACHE = {}


def _get_nc(scale: float):
    key = round(float(scale), 9)
    if key not in _BUILD_CACHE:
        _BUILD_CACHE[key] = build(float(scale))
    return _BUILD_CACHE[key]


def kernel(x1, x2, x3, x4=None, scale_factor=None, **_ignored):
    x1 = np.asarray(x1, dtype=np.float32)
    x2 = np.asarray(x2, dtype=np.float32)
    x3 = np.asarray(x3, dtype=np.float32)
    scale = float(np.asarray(scale_factor).reshape(-1)[0])

    # host prep: transpose Q,K to [d, s]; interleave V rows to [p, c*d]; round f32r
    qt = round_fp32r(x1.transpose(0, 2, 1))                     # [B, 128, SQ]
    kt = round_fp32r(x2.transpose(0, 2, 1))                     # [B, 128, SK]
    v = round_fp32r(
        x3.reshape(B, KC, 128, D).transpose(0, 2, 1, 3).reshape(B, 128, KC * D)
    )                                                           # [B, 128, SK]
    ones = np.ones((128, 128), dtype=np.float32)

    nc = _get_nc(scale)
    in_maps = []
    for c in range(NCORES):
        s = slice(c * BPC, (c + 1) * BPC)
        in_maps.append({
            "qt": np.ascontiguousarray(qt[s]),
            "kt": np.ascontiguousarray(kt[s]),
            "v": np.ascontiguousarray(v[s]),
            "ones": ones,
        })

    trace = bool(int(os.environ.get("KERNEL_TRACE", "0")))
    kwargs = {}
    if trace:
        _install_ntff_hook()
        if bool(int(os.environ.get("KERNEL_TRACE_ALL", "0"))):
            os.environ["BASS_PERFETTO_PROFILE_ALL_CORES"] = "1"
        kwargs = dict(trace=True, trace_kwargs={"title": "attention"})
    res = run_bass_kernel_spmd(nc, in_maps, core_ids=list(range(NCORES)), **kwargs)
    if trace:
        kernel.last_exec_ns = res.exec_time_ns
        kernel.last_trace = res.instructions_and_trace
        kernel.last_mean_exec_ns = res.mean_exec_time_ns

    outT = np.stack([r["o"] for r in res.results])              # [8, BPC, 128, SQ]
    out = outT.reshape(B, 128, SQ).transpose(0, 2, 1)           # [B, SQ, 128]
    return np.ascontiguousarray(out, dtype=np.float32)


kernel.last_exec_ns = None
kernel.last_trace = None
kernel.last_mean_exec_ns = None


# revision 18
# speedup vs baseline: 1.0461x; 1.0097x over previous
"""Trainium2 Bass kernel: batched scaled-dot-product attention.

reference: out[b] = softmax(scale * x1[b] @ x2[b].T) @ x3[b]
shapes: x1,x2,x3 = [16, 2048, 128] fp32.

Sharding: B=16 batches data-parallel over 8 NeuronCores (2 batches/core).

Device algorithm (per batch, per q-half of 1024):
  for k-chunk in 16 (128 K-rows each):
    S^T[k, q]   = matmul(lhsT=K^T chunk, rhs=Q^T half)    TensorE float32r
    eS          = exp(scale * S^T)     PSUM -> SBUF f32r  ScalarE
    outT[dv,q] += matmul(lhsT=V chunk, rhs=eS)            TensorE accumulate
    softmax denominators: partial sums of eS chunks accumulated on
    GPSIMD (chunks 0-5) and DVE (chunks 6-15)
  sums = ones-matmul(acc_g) + ones-matmul(acc_v)  (PSUM accumulate, bcast to
  all partitions);  rcp = approx 1/sums (DVE);  out^T = outT * rcp -> DMA.

Host side does layout only: pre-transpose Q,K; interleave V rows; pre-round
inputs to float32r (e8m11, bit-identical to the device cast); transpose the
output back. All FLOPs run on device.
"""
import os
import sys
import types
import numpy as np
from contextlib import ExitStack

import concourse.bass as bass  # noqa: F401
from concourse import bacc
import concourse.mybir as mybir
import concourse.tile as tile
from concourse.tile_rust import add_dep_helper
import concourse.bass_utils as bass_utils
from concourse.bass_utils import run_bass_kernel_spmd

f32 = mybir.dt.float32
f32r = mybir.dt.float32r

B, SQ, SK, D = 16, 2048, 2048, 128
NCORES = 8
BPC = B // NCORES  # batches per core
KC = SK // 128     # k chunks
NH = 2             # q halves
HW_ = SQ // NH     # 1024
GP_CHUNKS = 6      # sums chunks 0..GP_CHUNKS-1 on GPSIMD, rest on DVE


def _patch_ldw_opt():
    """Enable walrus LDWEIGHTS optimization (background weight-buffer overlap)."""
    if getattr(bass_utils, "_ldw_patched", False):
        return
    if not bool(int(os.environ.get("KERNEL_LDWOPT", "1"))):
        return
    orig = bass_utils.run_command

    def patched(argv, **kw):
        if isinstance(argv, list):
            argv = [
                "--enable-ldw-opt=true" if a == "--enable-ldw-opt=false" else a
                for a in argv
            ]
        return orig(argv, **kw)

    bass_utils.run_command = patched
    bass_utils._ldw_patched = True


def _install_ntff_hook():
    """Register the axon NTFF profile hook (used only when tracing)."""
    try:
        from antenv import axon_hooks  # noqa: F401
        return
    except ImportError:
        pass
    try:
        m = types.ModuleType("antenv.axon_hooks")
        m._hook = None
        m.set_axon_ntff_profile_hook = lambda h: setattr(m, "_hook", h)
        m.get_axon_ntff_profile_hook = lambda: m._hook
        sys.modules["antenv.axon_hooks"] = m
        import antenv
        antenv.axon_hooks = m
        from trn_agent_boot.trn_boot import _ntff_profile_via_ctypes
        m._hook = _ntff_profile_via_ctypes("/opt/axon/libaxon_pjrt.so")
    except Exception:
        pass


def round_fp32r(x: np.ndarray) -> np.ndarray:
    """RNE round fp32 -> float32r (e8m11: drop low 12 mantissa bits)."""
    u = np.ascontiguousarray(x, dtype=np.float32).view(np.uint32).astype(np.uint64)
    keep = 12
    lsb = (u >> keep) & 1
    bias = (1 << (keep - 1)) - 1 + lsb
    r = ((u + bias) & ~np.uint64((1 << keep) - 1)).astype(np.uint32)
    return r.view(np.float32)


def build(scale: float):
    _patch_ldw_opt()
    nc = bacc.Bacc("TRN2", target_bir_lowering=False, debug=False)
    qt = nc.dram_tensor("qt", [BPC, 128, SQ], f32r, kind="ExternalInput")
    kt = nc.dram_tensor("kt", [BPC, 128, SK], f32r, kind="ExternalInput")
    vv = nc.dram_tensor("v", [BPC, 128, SK], f32r, kind="ExternalInput")
    ones = nc.dram_tensor("ones", [128, 128], f32r, kind="ExternalInput")
    oo = nc.dram_tensor("o", [BPC, 128, SQ], f32, kind="ExternalOutput")

    Exp = mybir.ActivationFunctionType.Exp

    with tile.TileContext(nc) as tc, ExitStack() as ctx:
        inp = ctx.enter_context(tc.tile_pool(name="inp", bufs=2))
        es_pool = ctx.enter_context(tc.tile_pool(name="es", bufs=18))
        acc_pool = ctx.enter_context(tc.tile_pool(name="acc", bufs=2))
        out_pool = ctx.enter_context(tc.tile_pool(name="out", bufs=2))
        cpool = ctx.enter_context(tc.tile_pool(name="const", bufs=1))
        psS = ctx.enter_context(tc.tile_pool(name="psS", bufs=2, space="PSUM"))
        psO = ctx.enter_context(tc.tile_pool(name="psO", bufs=2, space="PSUM"))

        ones_sb = cpool.tile([128, 128], f32r, tag="ones")
        nc.sync.dma_start(ones_sb[:], ones.ap())

        def make_tail(ps_o, srcs, osl, out_dma):
            """Deferred per-half epilogue: sums bcast + recip + normalize.

            sums = ones @ (acc_gr + acc_vr + es15) accumulated in PSUM; es15
            itself is the third source so no add ever waits on the last exp.
            Emitted shortly into the NEXT half; `anchor` orders the matmuls
            behind that point in the PE stream."""
            def emit(anchors):
                ps_b = psS.tile([128, HW_], f32, tag="S")
                n_src = len(srcs)
                for si, srct in enumerate(srcs):
                    anchor = anchors[min(si, len(anchors) - 1)] if anchors else None
                    for j in range(HW_ // 512):
                        mm = nc.tensor.matmul(
                            ps_b[:, j * 512:(j + 1) * 512],
                            ones_sb[:],
                            srct[:, j * 512:(j + 1) * 512],
                            start=(si == 0), stop=(si == n_src - 1),
                        )
                        if anchor is not None:
                            add_dep_helper(mm.ins, anchor.ins, sync=False,
                                           reason="sums-mm deferred")
                rcp = acc_pool.tile([128, HW_], f32, tag="rcp")
                nc.vector.reciprocal_approx_fast(out=rcp[:], in_=ps_b[:])
                # fused normalize osl = (ps_o * 1.0) * rcp in 512-wide blocks
                # so each output DMA starts as soon as its half is ready
                for j in range(HW_ // 512):
                    jj = slice(j * 512, (j + 1) * 512)
                    nc.vector.scalar_tensor_tensor(
                        osl[:, jj], ps_o[:, jj], 1.0, rcp[:, jj],
                        op0=mybir.AluOpType.mult, op1=mybir.AluOpType.mult,
                    )
                    out_dma(j)
            return emit

        pending_tail = None

        def flush_tail(anchors):
            nonlocal pending_tail
            if pending_tail is not None:
                pending_tail(anchors)
                pending_tail = None

        for b in range(BPC):
            qt_sb = inp.tile([128, SQ], f32r, tag="qt")
            kt_sb = inp.tile([128, SK], f32r, tag="kt")
            v_sb = inp.tile([128, SK], f32r, tag="v")
            if b == 0:
                # cold start: critical-first fine-grained loads so chunk 0
                # can begin while the rest streams in
                qa, ka, va = qt.ap()[b], kt.ap()[b], vv.ap()[b]
                nc.sync.dma_start(kt_sb[:, 0:128], ka[:, 0:128])
                nc.sync.dma_start(qt_sb[:, 0:512], qa[:, 0:512])
                nc.sync.dma_start(qt_sb[:, 512:HW_], qa[:, 512:HW_])
                nc.sync.dma_start(kt_sb[:, 128:256], ka[:, 128:256])
                nc.sync.dma_start(v_sb[:, 0:128], va[:, 0:128])
                nc.sync.dma_start(kt_sb[:, 256:512], ka[:, 256:512])
                nc.sync.dma_start(v_sb[:, 128:512], va[:, 128:512])
                G = 4 * 128
                for g in range(1, KC * 128 // G):
                    sl = slice(g * G, (g + 1) * G)
                    nc.sync.dma_start(kt_sb[:, sl], ka[:, sl])
                    nc.sync.dma_start(v_sb[:, sl], va[:, sl])
                nc.sync.dma_start(qt_sb[:, HW_:SQ], qa[:, HW_:SQ])
            else:
                # prefetched during batch 0 compute: one DMA per tensor keeps
                # the wait structure on batch-1's first matmuls minimal
                nc.sync.dma_start(qt_sb[:], qt.ap()[b])
                nc.sync.dma_start(kt_sb[:], kt.ap()[b])
                nc.sync.dma_start(v_sb[:], vv.ap()[b])
            ot_sb = out_pool.tile([128, SQ], f32, tag="ot")

            for h in range(NH):
                q0 = h * HW_
                is_last_half = (b == BPC - 1 and h == NH - 1)
                raw_start = KC - 3 if is_last_half else KC - 1
                ps_o = psO.tile([128, HW_], f32, tag="psO")
                acc_g = acc_pool.tile([128, HW_], f32, tag="accg")
                acc_gr = acc_pool.tile([128, HW_], f32r, tag="accgr")
                acc_v = acc_pool.tile([128, HW_], f32, tag="accv")
                acc_vr = acc_pool.tile([128, HW_], f32r, tag="accvr")
                es_prev = None
                last_qk = None
                raw_es = []
                anchors = []
                for k in range(KC):
                    ps_s = psS.tile([128, HW_], f32, tag="S")
                    for j in range(HW_ // 512):
                        last_qk = nc.tensor.matmul(
                            ps_s[:, j * 512:(j + 1) * 512],
                            kt_sb[:, k * 128:(k + 1) * 128],
                            qt_sb[:, q0 + j * 512:q0 + (j + 1) * 512],
                            start=True, stop=True,
                        )
                    if k in (2, 3):
                        anchors.append(last_qk)
                    elif k == 4:
                        anchors.append(last_qk)
                        flush_tail(anchors)
                    es = es_pool.tile([128, HW_], f32r, tag="es")
                    nc.scalar.activation(es[:], ps_s[:], Exp, scale=scale)
                    for j in range(HW_ // 512):
                        nc.tensor.matmul(
                            ps_o[:, j * 512:(j + 1) * 512],
                            v_sb[:, k * 128:(k + 1) * 128],
                            es[:, j * 512:(j + 1) * 512],
                            start=(k == 0), stop=(k == KC - 1),
                        )
                    esf = es[:].bitcast(f32)
                    # softmax-denominator partials:
                    #   chunks 0-4  -> GPSIMD (front-loaded; last add f32r out)
                    #   chunks 5-14 -> DVE (last add f32r out)
                    #   chunk 15    -> fed raw (f32r) to the sums matmul
                    if k == 0 or k == GP_CHUNKS:
                        pass
                    elif k < GP_CHUNKS:
                        if k == 1:
                            nc.gpsimd.tensor_add(acc_g[:], es_prev, esf)
                        elif k == GP_CHUNKS - 1:
                            nc.gpsimd.tensor_add(acc_gr[:], acc_g[:], esf)
                        else:
                            nc.gpsimd.tensor_add(acc_g[:], acc_g[:], esf)
                    elif k == GP_CHUNKS + 1:
                        nc.vector.tensor_add(acc_v[:], es_prev, esf)
                    elif k == raw_start - 1:
                        nc.vector.tensor_add(acc_vr[:], acc_v[:], esf)
                    elif k >= raw_start:
                        raw_es.append(es)
                    else:
                        nc.vector.tensor_add(acc_v[:], acc_v[:], esf)
                    es_prev = esf

                dram_half = oo.ap()[b][:, q0:q0 + HW_]
                tile_half = ot_sb[:, q0:q0 + HW_]

                def out_dma(j, dram_half=dram_half, tile_half=tile_half):
                    jj = slice(j * 512, (j + 1) * 512)
                    nc.sync.dma_start(dram_half[:, jj], tile_half[:, jj])

                pending_tail = make_tail(
                    ps_o, [acc_gr, acc_vr] + [t[:] for t in raw_es],
                    tile_half, out_dma)

        # final half's epilogue + last output store
        flush_tail([])

    nc.compile()
    return nc


_BUILD_CACHE = {}


def _get_nc(scale: float):
    key = round(float(scale), 9)
    if key not in _BUILD_CACHE:
        _BUILD_CACHE[key] = build(float(scale))
    return _BUILD_CACHE[key]


def kernel(x1, x2, x3, x4=None, scale_factor=None, **_ignored):
    x1 = np.asarray(x1, dtype=np.float32)
    x2 = np.asarray(x2, dtype=np.float32)
    x3 = np.asarray(x3, dtype=np.float32)
    scale = float(np.asarray(scale_factor).reshape(-1)[0])

    # host prep: transpose Q,K to [d, s]; interleave V rows to [p, c*d]; round f32r
    qt = round_fp32r(x1.transpose(0, 2, 1))                     # [B, 128, SQ]
    kt = round_fp32r(x2.transpose(0, 2, 1))                     # [B, 128, SK]
    v = round_fp32r(
        x3.reshape(B, KC, 128, D).transpose(0, 2, 1, 3).reshape(B, 128, KC * D)
    )                                                           # [B, 128, SK]
    ones = np.ones((128, 128), dtype=np.float32)

    nc = _get_nc(scale)
    in_maps = []
    for c in range(NCORES):
        s = slice(c * BPC, (c + 1) * BPC)
        in_maps.append({
            "qt": np.ascontiguousarray(qt[s]),
            "kt": np.ascontiguousarray(kt[s]),
            "v": np.ascontiguousarray(v[s]),
            "ones": ones,
        })

    trace = bool(int(os.environ.get("KERNEL_TRACE", "0")))
    kwargs = {}
    if trace:
        _install_ntff_hook()
        if bool(int(os.environ.get("KERNEL_TRACE_ALL", "0"))):
            os.environ["BASS_PERFETTO_PROFILE_ALL_CORES"] = "1"
        kwargs = dict(trace=True, trace_kwargs={"title": "attention"})
    res = run_bass_kernel_spmd(nc, in_maps, core_ids=list(range(NCORES)), **kwargs)
    if trace:
        kernel.last_exec_ns = res.exec_time_ns
        kernel.last_trace = res.instructions_and_trace
        kernel.last_mean_exec_ns = res.mean_exec_time_ns

    outT = np.stack([r["o"] for r in res.results])              # [8, BPC, 128, SQ]
    out = outT.reshape(B, 128, SQ).transpose(0, 2, 1)           # [B, SQ, 128]
    return np.ascontiguousarray(out, dtype=np.float32)


kernel.last_exec_ns = None
kernel.last_trace = None
kernel.last_mean_exec_ns = None
